# revision 30
# baseline (speedup 1.0000x reference)
"""Trainium2 Bass kernel for nn_Attention_23364622090354.

Attention with RoPE + flat QK-RMSNorm + GQA (16 q heads, 4 kv heads) +
causal softmax. B=2, S=2048, DIM=2048, HD=128.

Sharding (8 NeuronCores = 2 batches x 4-way head tensor-parallel):
  core c -> batch b = c//4, head group g = c%4 (q heads 4g..4g+3, kv head g).
Every core runs the identical causal program (all 16 query tiles for its 4
heads), so the SPMD graph is uniform. Two tiny collectives per group of 4:
  - AllReduce of partial sum-of-squares rows (16KB) for the flattened-head
    RMSNorm of q (2048 dims) and k (512 dims);
  - AllToAll of the per-head attention output (2MB bf16) so each core runs
    the full output projection for its own 512 sequence rows (no output
    reduction needed).

Weights/x are pre-cast to bf16 and pre-transposed on the host; q/k head
dims are de-interleaved (even|odd -> lo|hi) so RoPE becomes two 64-partition
fused multiply-adds. The q/k norm gammas fold into a single per-(head,dim)
column multiplied into q alongside 1/rms.
"""
import copy

import numpy as np
import ml_dtypes

import concourse.bass as bass
import concourse.mybir as mybir
from concourse.tile import TileContext
from concourse.vector_clock import ScopedClock
from concourse import tile as _tile_mod

BF = ml_dtypes.bfloat16
F32, BF16 = mybir.dt.float32, mybir.dt.bfloat16

B, S, DIM = 2, 2048, 2048
NH, NKV, HD = 16, 4, 128
TP = 4
HPC = NH // TP            # q heads per core = 4
EPS = 1e-6
SCALE = float(HD) ** (-0.5)
NT = S // 128             # 16 token tiles
ND = DIM // 128           # 16 contraction tiles
TOK = S // TP             # 512 tokens owned per core after A2A

AluOp = mybir.AluOpType
AFT = mybir.ActivationFunctionType


# ---------------------------------------------------------------- patches --
_ws_counter = [0]


def _split_sync_waits(nc, limit=1):
    """This neuronxcc rejects >1 sem wait per instruction; move extras onto
    same-engine NoOps placed immediately before (engines run in order)."""
    tmpl = nc.sync.nop(nofuse=True, hint="waitsplit-template").ins
    for fn in nc.m.functions:
        for bb in fn.blocks:
            if tmpl in bb.instructions:
                bb.instructions.remove(tmpl)
    for fn in nc.m.functions:
        for bb in fn.blocks:
            out = []
            changed = False
            for inst in bb.instructions:
                si = inst.sync_info
                waits = list(si.on_wait) if si is not None and si.on_wait else []
                if len(waits) > limit:
                    for w in waits[:-limit]:
                        _ws_counter[0] += 1
                        nop = copy.copy(tmpl)
                        nop.name = f"I-waitsplit-{_ws_counter[0]}"
                        nop.engine = inst.engine
                        nop.sync_info = mybir.SyncInfo(on_wait=[w], on_update=[])
                        out.append(nop)
                    si.on_wait = waits[-limit:]
                    changed = True
                out.append(inst)
            if changed:
                try:
                    bb.instructions[:] = out
                except TypeError:
                    bb.instructions = out


def _patched_drain_and_barrier(self, tick_clock, wait_clock):
    """Kernel-tail drain with waits redistributed to 1-wait NOPs."""
    nc = self.nc
    probe = nc.sync.nop(nofuse=True, hint="drain_waits")
    wait_clock.add_sem_waits(probe.ins, ScopedClock({None: tick_clock.global_clock}))
    si = probe.ins.sync_info
    waits = list(si.on_wait or []) if si is not None else []
    if len(waits) > 1:
        si.on_wait = waits[:1]
        for w in waits[1:]:
            extra = nc.sync.nop(nofuse=True, hint="drain_waits")
            extra.ins.sync_info = mybir.SyncInfo(on_wait=[w], on_update=[])
    nc.sync.drain()
    nc.all_engine_barrier()
    assert self.sems is not None
    popped = nc._tile_sem_poison_stack.pop()
    assert popped is self._sem_poison
    nc.clear_and_free_semaphores(list(self.sems.allocated().values()))
    nc.all_engine_barrier()


_tile_mod.TileContext._drain_and_barrier = _patched_drain_and_barrier


# ------------------------------------------------------------------ graph --
def build_graph(debug=False):
    nc = bass.Bass()
    xt_d = nc.dram_tensor("xt", [DIM, S], BF16, kind="ExternalInput")
    wqt_d = nc.dram_tensor("wqt", [DIM, HPC * HD], BF16, kind="ExternalInput")
    wkt_d = nc.dram_tensor("wkt", [DIM, HD], BF16, kind="ExternalInput")
    wvt_d = nc.dram_tensor("wvt", [DIM, HD], BF16, kind="ExternalInput")
    wot_d = nc.dram_tensor("wot", [NH * HD, DIM], BF16, kind="ExternalInput")
    fq_d = nc.dram_tensor("fq", [128, 4, S], BF16, kind="ExternalInput")
    wcol_d = nc.dram_tensor("wcol", [HD, HPC], F32, kind="ExternalInput")
    masks_d = nc.dram_tensor("masks", [128, 4, 512], BF16, kind="ExternalInput")
    bsel_d = nc.dram_tensor("bsel", [128, 2], F32, kind="ExternalInput")
    out_d = nc.dram_tensor("out", [TOK, DIM], F32, kind="ExternalOutput")
    dbg = {}
    if debug:
        dbg["qt"] = nc.dram_tensor("dbg_qt", [128, HPC, S], BF16, kind="ExternalOutput")
        dbg["kt"] = nc.dram_tensor("dbg_kt", [128, S], BF16, kind="ExternalOutput")
        dbg["v"] = nc.dram_tensor("dbg_v", [128, NT, HD], BF16, kind="ExternalOutput")
        dbg["ssq"] = nc.dram_tensor("dbg_ssq", [2, S], F32, kind="ExternalOutput")
        dbg["at"] = nc.dram_tensor("dbg_at", [128, NH, 512], BF16,
                                   kind="ExternalOutput")

    groups4 = [[0, 1, 2, 3], [4, 5, 6, 7]]
    groups8 = [list(range(8))]

    from contextlib import ExitStack
    with TileContext(nc) as tc, ExitStack() as outer:
        consts = outer.enter_context(tc.tile_pool(name="consts", bufs=1))
        dram = outer.enter_context(tc.tile_pool(name="dram", bufs=1, space="DRAM"))

        fq_sb = consts.tile([128, 4, S], BF16)
        masks_sb = consts.tile([128, 4, 512], BF16)
        wcol_sb = consts.tile([HD, HPC], F32)
        bsel_sb = consts.tile([128, 2], F32)
        nc.sync.dma_start(out=bsel_sb, in_=bsel_d[:, :])
        ones_col = consts.tile([128, 1], F32)
        nc.vector.memset(ones_col, 1.0)
        ones_colb = consts.tile([128, 1], BF16)
        nc.vector.memset(ones_colb, 1.0)
        ones_row = consts.tile([1, 128], F32)
        nc.vector.memset(ones_row, 1.0)
        eps_sb = consts.tile([1, 1], F32)
        nc.vector.memset(eps_sb, EPS)

        # per-head AllToAll buffers (8-core mesh; see _prep for the
        # batch-duplication scheme)
        a2a_in = [dram.tile([2 * TP, HD, 512], BF16, name=f"a2a_in{h}", tag=f"a2a_in{h}")
                  for h in range(HPC)]
        a2a_out = [dram.tile([2 * TP, HD, 512], BF16, name=f"a2a_out{h}", tag=f"a2a_out{h}")
                   for h in range(HPC)]
        ssq_in = [dram.tile([1, 2, 512], F32, name=f"ssq_in{t}",
                            tag=f"ssq_in{t}") for t in range(4)]
        ssq_out = [dram.tile([1, 2, 512], F32, name=f"ssq_out{t}",
                             tag=f"ssq_out{t}") for t in range(4)]

        persist = outer.enter_context(tc.tile_pool(name="persist", bufs=1))
        qt_f = persist.tile([128, HPC, S], BF16)
        kt_f = persist.tile([128, S], BF16)
        v_sb = persist.tile([128, NT, HD], BF16)

        # ---------------- P1: q/k projections + rope, ssq, AllReduce -------
        with ExitStack() as p1s:
            p1p = p1s.enter_context(tc.tile_pool(name="p1p", bufs=1))
            xtp = p1s.enter_context(tc.tile_pool(name="xtp", bufs=24))
            p1t = p1s.enter_context(tc.tile_pool(name="p1t", bufs=3))
            rows = p1s.enter_context(tc.tile_pool(name="rows", bufs=1))
            qn_pool = p1s.enter_context(tc.tile_pool(name="qn", bufs=1))
            pp = p1s.enter_context(tc.tile_pool(name="pp", bufs=2, space="PSUM"))
            ppv = p1s.enter_context(tc.tile_pool(name="ppv", bufs=2, space="PSUM"))
            pps = p1s.enter_context(tc.tile_pool(name="pps", bufs=2, space="PSUM"))

            wq_sb = p1p.tile([128, ND, HPC * HD], BF16)
            wk_sb = p1p.tile([128, ND, HD], BF16)
            wv_sb = p1p.tile([128, ND, HD], BF16)
            for dt in range(ND):
                rsl = slice(128 * dt, 128 * (dt + 1))
                nc.sync.dma_start(out=wq_sb[:, dt, :], in_=wqt_d[rsl, :])
                nc.sync.dma_start(out=wk_sb[:, dt, :], in_=wkt_d[rsl, :])
                nc.sync.dma_start(out=wv_sb[:, dt, :], in_=wvt_d[rsl, :])

            for fc4 in range(4):
                nc.sync.dma_start(out=fq_sb[:, fc4, :], in_=fq_d[:, fc4, :])
            nc.sync.dma_start(out=masks_sb, in_=masks_d[:, :, :])
            nc.sync.dma_start(out=wcol_sb, in_=wcol_d[:, :])

            qt_n = qn_pool.tile([128, HPC, S], BF16)   # roped, pre-norm
            kt_n = qn_pool.tile([128, S], BF16)
            rq_bc = qn_pool.tile([128, S], F32)
            rk_bc = qn_pool.tile([128, S], F32)
            sq2 = rows.tile([1, 2, S], F32, tag="rowA")

            def load_xtt(cols):
                xtt = []
                for dt in range(ND):
                    xt_tile = xtp.tile([128, 512], BF16, tag="xt")
                    nc.sync.dma_start(
                        out=xt_tile, in_=xt_d[128 * dt:128 * (dt + 1), cols])
                    xtt.append(xt_tile)
                return xtt

            def rope_emit(ps, dst, cols):
                # ACT evacuates psum; DVE multiplies from SBUF (faster port)
                ev = p1t.tile([128, 512], F32, tag="ev")
                nc.scalar.copy(out=ev, in_=ps)
                for half, (ca, cb) in enumerate(((0, 1), (2, 3))):
                    ta = p1t.tile([64, 512], F32, tag="ropeA")
                    tb = p1t.tile([64, 512], F32, tag="ropeB")
                    nc.vector.tensor_mul(ta, ev[0:64, :],
                                         fq_sb[0:64, ca, cols])
                    nc.gpsimd.tensor_mul(tb, ev[64:128, :],
                                         fq_sb[64:128, cb, cols])
                    nc.vector.tensor_tensor(
                        out=dst[64 * half:64 * (half + 1), cols],
                        in0=ta, in1=tb, op=AluOp.add)

            for t4 in range(4):
                cols = slice(512 * t4, 512 * (t4 + 1))
                xtt = load_xtt(cols)
                for h in range(HPC):
                    ps = pp.tile([128, 512], F32)
                    for dt in range(ND):
                        nc.tensor.matmul(
                            ps, wq_sb[:, dt, HD * h:HD * (h + 1)], xtt[dt],
                            start=(dt == 0), stop=(dt == ND - 1))
                    rope_emit(ps, qt_n[:, h], cols)
                ps = pp.tile([128, 512], F32)
                for dt in range(ND):
                    nc.tensor.matmul(ps, wk_sb[:, dt, :], xtt[dt],
                                     start=(dt == 0), stop=(dt == ND - 1))
                rope_emit(ps, kt_n, cols)
                for tt in range(4):
                    psv = ppv.tile([128, HD], F32)
                    for dt in range(ND):
                        nc.tensor.matmul(
                            psv, xtt[dt][:, 128 * tt:128 * (tt + 1)],
                            wv_sb[:, dt, :],
                            start=(dt == 0), stop=(dt == ND - 1))
                    nc.scalar.copy(out=v_sb[:, 4 * t4 + tt, :], in_=psv)

                # per-chunk flat-head ssq partials -> 4-core AllReduce ->
                # rstd -> broadcast -> normalize; pipelines under later chunks
                sps = pps.tile([1, 512], F32, name="sps", tag="ssqp")
                for h in range(HPC):
                    sq = p1t.tile([128, 512], F32, tag="sq")
                    nc.gpsimd.tensor_mul(sq, qt_n[:, h, cols], qt_n[:, h, cols])
                    nc.tensor.matmul(sps, ones_col, sq,
                                     start=(h == 0), stop=(h == HPC - 1))
                sq2 = rows.tile([1, 2, 512], F32, tag="sq2", name="sq2", bufs=2)
                nc.vector.tensor_copy(sq2[:, 0, :], sps)
                sps_k = pps.tile([1, 512], F32, name="sps_k", tag="ssqp")
                sqk = p1t.tile([128, 512], F32, tag="sq")
                nc.gpsimd.tensor_mul(sqk, kt_n[:, cols], kt_n[:, cols])
                nc.tensor.matmul(sps_k, ones_col, sqk, start=True, stop=True)
                nc.vector.tensor_copy(sq2[:, 1, :], sps_k)

                nc.sync.dma_start(out=ssq_in[t4][:, :, :], in_=sq2)
                nc.gpsimd.collective_compute(
                    "AllReduce", AluOp.add, replica_groups=groups4,
                    ins=[ssq_in[t4].opt()], outs=[ssq_out[t4].opt()])
                rs_sb = rows.tile([1, 2, 512], F32, tag="rs", name="rs", bufs=2)
                nc.sync.dma_start(out=rs_sb, in_=ssq_out[t4][:, :, :])
                if debug:
                    nc.sync.dma_start(out=dbg["ssq"][:, cols],
                                      in_=rs_sb.rearrange("p r s -> (p r) s"))
                tmp2 = rows.tile([1, 2, 512], F32, tag="tmp", name="tmp", bufs=2)
                rr2 = rows.tile([1, 2, 512], F32, tag="rr", name="rr", bufs=2)
                nc.scalar.activation(out=tmp2[:, 0, :], in_=rs_sb[:, 0, :],
                                     func=AFT.Sqrt, scale=1.0 / (NH * HD),
                                     bias=eps_sb)
                nc.scalar.activation(out=tmp2[:, 1, :], in_=rs_sb[:, 1, :],
                                     func=AFT.Sqrt, scale=1.0 / (NKV * HD),
                                     bias=eps_sb)
                nc.vector.reciprocal(rr2[:, 0, :], tmp2[:, 0, :])
                nc.vector.reciprocal(rr2[:, 1, :], tmp2[:, 1, :])
                bq = pps.tile([128, 512], F32, tag="bcq")
                nc.tensor.matmul(bq, ones_row, rr2[:, 0, :],
                                 start=True, stop=True)
                nc.vector.tensor_copy(rq_bc[:, cols], bq)
                bk = pps.tile([128, 512], F32, tag="bcq")
                nc.tensor.matmul(bk, ones_row, rr2[:, 1, :],
                                 start=True, stop=True)
                nc.vector.tensor_copy(rk_bc[:, cols], bk)
                for h in range(HPC):
                    nc.vector.scalar_tensor_tensor(
                        out=qt_f[:, h, cols], in0=qt_n[:, h, cols],
                        scalar=wcol_sb[:, h:h + 1], in1=rq_bc[:, cols],
                        op0=AluOp.mult, op1=AluOp.mult)
                nc.vector.tensor_tensor(out=kt_f[:, cols], in0=kt_n[:, cols],
                                        in1=rk_bc[:, cols], op=AluOp.mult)

        if debug:
            nc.sync.dma_start(out=dbg["qt"][:, :, :], in_=qt_f)
            nc.sync.dma_start(out=dbg["kt"][:, :], in_=kt_f)
            nc.sync.dma_start(out=dbg["v"][:, :, :], in_=v_sb)

        # wo prefetch pool: opened after P1 pools close, survives into P5
        wop = outer.enter_context(tc.tile_pool(name="wop", bufs=1))
        wo_sb = wop.tile([128, ND, DIM], BF16)
        nc.sync.dma_start(out=wo_sb,
                          in_=wot_d.rearrange("(n p) e -> p n e", p=128))

        # ---------------- P3: attention (h outer; per-head A2A) ------------
        with ExitStack() as p3s:
            epool = p3s.enter_context(tc.tile_pool(name="epool", bufs=2))
            eraw_p = p3s.enter_context(tc.tile_pool(name="eraw", bufs=3))
            smal = p3s.enter_context(tc.tile_pool(name="smal", bufs=3))
            atp = p3s.enter_context(tc.tile_pool(name="atp", bufs=3))
            stps = p3s.enter_context(
                tc.tile_pool(name="stps", bufs=3, space="PSUM"))
            bcp3 = p3s.enter_context(
                tc.tile_pool(name="bcp3", bufs=1, space="PSUM"))
            pvps = p3s.enter_context(
                tc.tile_pool(name="pvps", bufs=2, space="PSUM"))
            dnps = p3s.enter_context(
                tc.tile_pool(name="dnps", bufs=2, space="PSUM"))
            for h in range(HPC):
                for m in range(4):
                    qcols = slice(512 * m, 512 * (m + 1))
                    nkb = 4 * m + 4
                    et = epool.tile([128, NT, 512], BF16, tag="e")
                    dn = dnps.tile([1, 512], F32)
                    # all score matmuls first: keeps the in-order PE queue
                    # free of cross-engine waits (exp) between STs
                    for kb in range(nkb):
                        st = stps.tile([128, 512], F32, name="st", tag="st")
                        nc.tensor.matmul(
                            st, kt_f[:, 128 * kb:128 * (kb + 1)],
                            qt_f[:, h, qcols], start=True, stop=True)
                        if kb >= 4 * m:
                            o = kb - 4 * m
                            w = 512 - 128 * o
                            er = eraw_p.tile([128, 512], BF16, tag="eraw")
                            nc.scalar.activation(out=er[:, 0:w],
                                                 in_=st[:, 512 - w:512],
                                                 func=AFT.Exp, scale=SCALE)
                            nc.vector.tensor_mul(
                                et[:, kb, 512 - w:512], er[:, 0:w],
                                masks_sb[:, o, 512 - w:512])
                            if o > 0:
                                nc.gpsimd.memset(et[:, kb, 0:512 - w], 0.0)
                        else:
                            nc.scalar.activation(out=et[:, kb, :], in_=st,
                                                 func=AFT.Exp, scale=SCALE)
                    for kb in range(nkb):
                        nc.tensor.matmul(dn, ones_colb, et[:, kb, :],
                                         start=(kb == 0), stop=(kb == nkb - 1))
                    rd = smal.tile([1, 512], F32, tag="rd")
                    nc.vector.reciprocal(rd, dn)
                    bc = bcp3.tile([128, 512], F32)
                    nc.tensor.matmul(bc, ones_row, rd, start=True, stop=True)
                    rdb = smal.tile([128, 512], F32, tag="rdb")
                    nc.vector.tensor_copy(rdb, bc)
                    at_ps = pvps.tile([128, 512], F32)
                    for kb in range(nkb):
                        nc.tensor.matmul(at_ps, v_sb[:, kb, :], et[:, kb, :],
                                         start=(kb == 0), stop=(kb == nkb - 1))
                    at0 = atp.tile([128, 512], BF16, tag="at0")
                    at1 = atp.tile([128, 512], BF16, tag="at1")
                    nc.vector.scalar_tensor_tensor(
                        out=at0, in0=at_ps, scalar=bsel_sb[:, 0:1], in1=rdb,
                        op0=AluOp.mult, op1=AluOp.mult)
                    nc.vector.scalar_tensor_tensor(
                        out=at1, in0=at_ps, scalar=bsel_sb[:, 1:2], in1=rdb,
                        op0=AluOp.mult, op1=AluOp.mult)
                    nc.sync.dma_start(out=a2a_in[h][m, :, :], in_=at0)
                    nc.sync.dma_start(out=a2a_in[h][TP + m, :, :], in_=at1)
                nc.gpsimd.collective_compute(
                    "AllToAll", AluOp.bypass, replica_groups=groups8,
                    ins=[a2a_in[h].opt()], outs=[a2a_out[h].opt()])

        # ---------------- P5: output projection ----------------
        with ExitStack() as p5s:
            p5 = p5s.enter_context(tc.tile_pool(name="p5", bufs=1))
            p5t = p5s.enter_context(tc.tile_pool(name="p5t", bufs=3))
            pop = p5s.enter_context(
                tc.tile_pool(name="pop", bufs=2, space="PSUM"))
            gt_sb = p5.tile([128, NH, 512], BF16)
            for h in range(HPC):
                for i in range(TP):
                    sA = p5t.tile([128, 512], BF16, tag="sA")
                    sB = p5t.tile([128, 512], BF16, tag="sB")
                    nc.sync.dma_start(out=sA, in_=a2a_out[h][i, :, :])
                    nc.sync.dma_start(out=sB, in_=a2a_out[h][TP + i, :, :])
                    nc.vector.tensor_tensor(out=gt_sb[:, 4 * i + h, :],
                                            in0=sA, in1=sB, op=AluOp.add)
            if debug:
                nc.sync.dma_start(out=dbg["at"][:, :, :], in_=gt_sb)
            e16_order = [4 * i + h for h in range(HPC) for i in range(TP)]
            for tt in range(4):
                opsd = [pop.tile([128, 512], F32, name=f"ops{d}", tag=f"dc{d}")
                        for d in range(4)]
                for idx, e16 in enumerate(e16_order):
                    for dc in range(4):
                        nc.tensor.matmul(
                            opsd[dc], gt_sb[:, e16, 128 * tt:128 * (tt + 1)],
                            wo_sb[:, e16, 512 * dc:512 * (dc + 1)],
                            start=(idx == 0), stop=(idx == NH - 1))
                for dc in range(4):
                    osb = p5t.tile([128, 512], F32, tag="osb")
                    nc.vector.tensor_copy(osb, opsd[dc])
                    nc.sync.dma_start(
                        out=out_d[128 * tt:128 * (tt + 1),
                                  512 * dc:512 * (dc + 1)],
                        in_=osb)

    _split_sync_waits(nc)
    return nc


# ------------------------------------------------------------- host side --
_PERM = np.concatenate([np.arange(0, HD, 2), np.arange(1, HD, 2)])


def _prep(inputs):
    x = np.asarray(inputs["x"], np.float32)
    fc = np.asarray(inputs["freq_cis"], np.float32)
    wq = np.asarray(inputs["wq"], np.float32)
    wk = np.asarray(inputs["wk"], np.float32)
    wv = np.asarray(inputs["wv"], np.float32)
    wo = np.asarray(inputs["wo"], np.float32)
    qnw = np.asarray(inputs["q_norm_w"], np.float32)
    knw = np.asarray(inputs["k_norm_w"], np.float32)

    wq_p = wq.reshape(NH, HD, DIM)[:, _PERM, :]
    wk_p = wk.reshape(NKV, HD, DIM)[:, _PERM, :]
    qnw_p = qnw.reshape(NH, HD)[:, _PERM]
    knw_p = knw.reshape(NKV, HD)[:, _PERM]

    xt = [np.ascontiguousarray(x[b].T).astype(BF) for b in range(B)]
    wqt = [np.ascontiguousarray(
        wq_p[4 * g:4 * (g + 1)].reshape(4 * HD, DIM).T).astype(BF)
        for g in range(TP)]
    wkt = [np.ascontiguousarray(wk_p[g].T).astype(BF) for g in range(TP)]
    wvt = [np.ascontiguousarray(wv[g * HD:(g + 1) * HD].T).astype(BF)
           for g in range(TP)]
    wot = np.ascontiguousarray(wo.T).astype(BF)

    fq = np.stack([fc[:, :, 0, 0].T, fc[:, :, 0, 1].T,
                   fc[:, :, 1, 0].T, fc[:, :, 1, 1].T], axis=1)
    fq = np.concatenate([fq, fq], axis=0)                 # both partition halves
    fq = np.ascontiguousarray(fq).astype(BF)              # [128, 4, S]

    wcol = []
    for g in range(TP):
        cols = np.empty((HD, HPC), np.float32)
        for hl in range(HPC):
            cols[:, hl] = qnw_p[4 * g + hl] * knw_p[g]
        wcol.append(np.ascontiguousarray(cols))

    k_idx = np.arange(128)[:, None]
    q_idx = np.arange(512)[None, :]
    masks = np.stack([(o * 128 + k_idx <= q_idx) for o in range(4)],
                     axis=1).astype(BF)                   # [128, 4, 512]
    masks = np.ascontiguousarray(masks)

    in_maps = []
    for c in range(8):
        b, g = divmod(c, TP)
        bsel = np.zeros((128, 2), np.float32)
        bsel[:, b] = 1.0
        in_maps.append({
            "xt": xt[b], "wqt": wqt[g], "wkt": wkt[g], "wvt": wvt[g],
            "wot": wot, "fq": fq, "wcol": wcol[g], "masks": masks,
            "bsel": bsel,
        })
    return in_maps


_GRAPH_CACHE = {}


def _get_graph(debug=False):
    key = bool(debug)
    if key not in _GRAPH_CACHE:
        _GRAPH_CACHE[key] = build_graph(debug=key)
    return _GRAPH_CACHE[key]


LAST_RESULT = None


def kernel(debug=False, _run_kwargs=None, **inputs):
    global LAST_RESULT
    from concourse.bass_utils import run_bass_kernel_spmd

    nc = _get_graph(debug=debug)
    in_maps = _prep(inputs)
    res = run_bass_kernel_spmd(nc, in_maps, core_ids=list(range(8)),
                               **(_run_kwargs or {}))
    LAST_RESULT = res
    out = np.empty((B, S, DIM), np.float32)
    for c in range(8):
        b, g = divmod(c, TP)
        out[b, TOK * g:TOK * (g + 1), :] = res.results[c]["out"]
    if debug:
        return out, res
    return out


# revision 33
# speedup vs baseline: 1.1642x; 1.1642x over previous
"""Trainium2 Bass kernel for nn_Attention_23364622090354.

Attention with RoPE + flat QK-RMSNorm + GQA (16 q heads, 4 kv heads) +
causal softmax. B=2, S=2048, DIM=2048, HD=128.

Sharding (8 NeuronCores = 2 batches x 4-way head tensor-parallel):
  core c -> batch b = c//4, head group g = c%4 (q heads 4g..4g+3, kv head g).
Every core runs the identical causal program (all 16 query tiles for its 4
heads), so the SPMD graph is uniform. Two tiny collectives per group of 4:
  - AllReduce of partial sum-of-squares rows (16KB) for the flattened-head
    RMSNorm of q (2048 dims) and k (512 dims);
  - AllToAll of the per-head attention output (2MB bf16) so each core runs
    the full output projection for its own 512 sequence rows (no output
    reduction needed).

Weights/x are pre-cast to bf16 and pre-transposed on the host; q/k head
dims are de-interleaved (even|odd -> lo|hi) so RoPE becomes two 64-partition
fused multiply-adds. The q/k norm gammas fold into a single per-(head,dim)
column multiplied into q alongside 1/rms.
"""
import copy

import numpy as np
import ml_dtypes

import concourse.bass as bass
import concourse.mybir as mybir
from concourse.tile import TileContext
from concourse.vector_clock import ScopedClock
from concourse import tile as _tile_mod

BF = ml_dtypes.bfloat16
F32, BF16 = mybir.dt.float32, mybir.dt.bfloat16

B, S, DIM = 2, 2048, 2048
NH, NKV, HD = 16, 4, 128
TP = 4
HPC = NH // TP            # q heads per core = 4
EPS = 1e-6
SCALE = float(HD) ** (-0.5)
NT = S // 128             # 16 token tiles
ND = DIM // 128           # 16 contraction tiles
TOK = S // TP             # 512 tokens owned per core after A2A

AluOp = mybir.AluOpType
AFT = mybir.ActivationFunctionType


# ---------------------------------------------------------------- patches --
_ws_counter = [0]


def _split_sync_waits(nc, limit=1):
    """This neuronxcc rejects >1 sem wait per instruction; move extras onto
    same-engine NoOps placed immediately before (engines run in order)."""
    tmpl = nc.sync.nop(nofuse=True, hint="waitsplit-template").ins
    for fn in nc.m.functions:
        for bb in fn.blocks:
            if tmpl in bb.instructions:
                bb.instructions.remove(tmpl)
    for fn in nc.m.functions:
        for bb in fn.blocks:
            out = []
            changed = False
            for inst in bb.instructions:
                si = inst.sync_info
                waits = list(si.on_wait) if si is not None and si.on_wait else []
                if len(waits) > limit:
                    for w in waits[:-limit]:
                        _ws_counter[0] += 1
                        nop = copy.copy(tmpl)
                        nop.name = f"I-waitsplit-{_ws_counter[0]}"
                        nop.engine = inst.engine
                        nop.sync_info = mybir.SyncInfo(on_wait=[w], on_update=[])
                        out.append(nop)
                    si.on_wait = waits[-limit:]
                    changed = True
                out.append(inst)
            if changed:
                try:
                    bb.instructions[:] = out
                except TypeError:
                    bb.instructions = out


def _patched_drain_and_barrier(self, tick_clock, wait_clock):
    """Kernel-tail drain with waits redistributed to 1-wait NOPs."""
    nc = self.nc
    probe = nc.sync.nop(nofuse=True, hint="drain_waits")
    wait_clock.add_sem_waits(probe.ins, ScopedClock({None: tick_clock.global_clock}))
    si = probe.ins.sync_info
    waits = list(si.on_wait or []) if si is not None else []
    if len(waits) > 1:
        si.on_wait = waits[:1]
        for w in waits[1:]:
            extra = nc.sync.nop(nofuse=True, hint="drain_waits")
            extra.ins.sync_info = mybir.SyncInfo(on_wait=[w], on_update=[])
    nc.sync.drain()
    nc.all_engine_barrier()
    assert self.sems is not None
    popped = nc._tile_sem_poison_stack.pop()
    assert popped is self._sem_poison
    nc.clear_and_free_semaphores(list(self.sems.allocated().values()))
    nc.all_engine_barrier()


_tile_mod.TileContext._drain_and_barrier = _patched_drain_and_barrier


# ------------------------------------------------------------------ graph --
def build_graph(debug=False):
    nc = bass.Bass()
    xt_d = nc.dram_tensor("xt", [DIM, S], BF16, kind="ExternalInput")
    wqt_d = nc.dram_tensor("wqt", [DIM, HPC * HD], BF16, kind="ExternalInput")
    wkt_d = nc.dram_tensor("wkt", [DIM, HD], BF16, kind="ExternalInput")
    wvt_d = nc.dram_tensor("wvt", [DIM, HD], BF16, kind="ExternalInput")
    wot_d = nc.dram_tensor("wot", [NH * HD, DIM], BF16, kind="ExternalInput")
    fq_d = nc.dram_tensor("fq", [128, 4, S], BF16, kind="ExternalInput")
    wcol_d = nc.dram_tensor("wcol", [HD, HPC], F32, kind="ExternalInput")
    masks_d = nc.dram_tensor("masks", [128, 4, 512], BF16, kind="ExternalInput")
    bsel_d = nc.dram_tensor("bsel", [128, 2], F32, kind="ExternalInput")
    out_d = nc.dram_tensor("out", [TOK, DIM], F32, kind="ExternalOutput")
    dbg = {}
    if debug:
        dbg["qt"] = nc.dram_tensor("dbg_qt", [128, HPC, S], BF16, kind="ExternalOutput")
        dbg["kt"] = nc.dram_tensor("dbg_kt", [128, S], BF16, kind="ExternalOutput")
        dbg["v"] = nc.dram_tensor("dbg_v", [128, NT, HD], BF16, kind="ExternalOutput")
        dbg["ssq"] = nc.dram_tensor("dbg_ssq", [2, S], F32, kind="ExternalOutput")
        dbg["at"] = nc.dram_tensor("dbg_at", [128, NH, 512], BF16,
                                   kind="ExternalOutput")

    groups4 = [[0, 1, 2, 3], [4, 5, 6, 7]]
    groups8 = [list(range(8))]

    from contextlib import ExitStack
    with TileContext(nc) as tc, ExitStack() as outer:
        consts = outer.enter_context(tc.tile_pool(name="consts", bufs=1))
        dram = outer.enter_context(tc.tile_pool(name="dram", bufs=1, space="DRAM"))

        fq_sb = consts.tile([128, 4, S], BF16)
        masks_sb = consts.tile([128, 4, 512], BF16)
        wcol_sb = consts.tile([HD, HPC], F32)
        bsel_sb = consts.tile([128, 2], F32)
        nc.sync.dma_start(out=bsel_sb, in_=bsel_d[:, :])
        ones_col = consts.tile([128, 1], F32)
        nc.vector.memset(ones_col, 1.0)
        ones_colb = consts.tile([128, 1], BF16)
        nc.vector.memset(ones_colb, 1.0)
        ones_row = consts.tile([1, 128], F32)
        nc.vector.memset(ones_row, 1.0)
        eps_sb = consts.tile([1, 1], F32)
        nc.vector.memset(eps_sb, EPS)

        # per-head AllToAll buffers (8-core mesh; see _prep for the
        # batch-duplication scheme)
        a2a_in = [dram.tile([2 * TP, HD, 512], BF16, name=f"a2a_in{h}", tag=f"a2a_in{h}")
                  for h in range(HPC)]
        a2a_out = [dram.tile([2 * TP, HD, 512], BF16, name=f"a2a_out{h}", tag=f"a2a_out{h}")
                   for h in range(HPC)]
        ssq_in = [dram.tile([1, 2, 512], F32, name=f"ssq_in{t}",
                            tag=f"ssq_in{t}") for t in range(4)]
        ssq_out = [dram.tile([1, 2, 512], F32, name=f"ssq_out{t}",
                             tag=f"ssq_out{t}") for t in range(4)]

        persist = outer.enter_context(tc.tile_pool(name="persist", bufs=1))
        qt_f = persist.tile([128, HPC, S], BF16)
        kt_f = persist.tile([128, S], BF16)
        v_sb = persist.tile([128, NT, HD], BF16)

        # ---------------- P1: q/k projections + rope, ssq, AllReduce -------
        with ExitStack() as p1s:
            p1p = p1s.enter_context(tc.tile_pool(name="p1p", bufs=1))
            xtp = p1s.enter_context(tc.tile_pool(name="xtp", bufs=36))
            p1t = p1s.enter_context(tc.tile_pool(name="p1t", bufs=3))
            rows = p1s.enter_context(tc.tile_pool(name="rows", bufs=1))
            qn_pool = p1s.enter_context(tc.tile_pool(name="qn", bufs=1))
            pp = p1s.enter_context(tc.tile_pool(name="pp", bufs=2, space="PSUM"))
            ppv = p1s.enter_context(tc.tile_pool(name="ppv", bufs=2, space="PSUM"))
            pps = p1s.enter_context(tc.tile_pool(name="pps", bufs=2, space="PSUM"))

            wq_sb = p1p.tile([128, ND, HPC * HD], BF16)
            wk_sb = p1p.tile([128, ND, HD], BF16)
            wv_sb = p1p.tile([128, ND, HD], BF16)
            for dt in range(ND):
                rsl = slice(128 * dt, 128 * (dt + 1))
                nc.sync.dma_start(out=wq_sb[:, dt, :], in_=wqt_d[rsl, :])
                nc.sync.dma_start(out=wk_sb[:, dt, :], in_=wkt_d[rsl, :])
                nc.sync.dma_start(out=wv_sb[:, dt, :], in_=wvt_d[rsl, :])

            for fc4 in range(4):
                nc.sync.dma_start(out=fq_sb[:, fc4, :], in_=fq_d[:, fc4, :])
            nc.sync.dma_start(out=masks_sb, in_=masks_d[:, :, :])
            nc.sync.dma_start(out=wcol_sb, in_=wcol_d[:, :])

            qt_n = qn_pool.tile([128, HPC, S], BF16)   # roped, pre-norm
            kt_n = qn_pool.tile([128, S], BF16)
            rq_bc = qn_pool.tile([128, S], F32)
            rk_bc = qn_pool.tile([128, S], F32)
            sq2 = rows.tile([1, 2, S], F32, tag="rowA")

            def load_xtt(cols):
                xtt = []
                for dt in range(ND):
                    xt_tile = xtp.tile([128, 512], BF16, tag="xt")
                    nc.sync.dma_start(
                        out=xt_tile, in_=xt_d[128 * dt:128 * (dt + 1), cols])
                    xtt.append(xt_tile)
                return xtt

            def rope_emit(ps, dst, cols):
                # ACT evacuates psum; DVE multiplies from SBUF (faster port)
                ev = p1t.tile([128, 512], F32, tag="ev")
                nc.scalar.copy(out=ev, in_=ps)
                for half, (ca, cb) in enumerate(((0, 1), (2, 3))):
                    ta = p1t.tile([64, 512], F32, tag="ropeA")
                    tb = p1t.tile([64, 512], F32, tag="ropeB")
                    nc.vector.tensor_mul(ta, ev[0:64, :],
                                         fq_sb[0:64, ca, cols])
                    nc.vector.tensor_mul(tb, ev[64:128, :],
                                         fq_sb[64:128, cb, cols])
                    nc.vector.tensor_tensor(
                        out=dst[64 * half:64 * (half + 1), cols],
                        in0=ta, in1=tb, op=AluOp.add)

            def v_proj(t4, xtt):
                for tt in range(4):
                    psv = ppv.tile([128, HD], F32)
                    for dt in range(ND):
                        nc.tensor.matmul(
                            psv, xtt[dt][:, 128 * tt:128 * (tt + 1)],
                            wv_sb[:, dt, :],
                            start=(dt == 0), stop=(dt == ND - 1))
                    nc.scalar.copy(out=v_sb[:, 4 * t4 + tt, :], in_=psv)

            def ssq_ar(t4):
                # per-chunk flat-head ssq partials -> 4-core AllReduce
                cols = slice(512 * t4, 512 * (t4 + 1))
                sps = pps.tile([1, 512], F32, name="sps", tag="ssqp")
                for h in range(HPC):
                    sq = p1t.tile([128, 512], F32, tag="sq")
                    nc.gpsimd.tensor_mul(sq, qt_n[:, h, cols], qt_n[:, h, cols])
                    nc.tensor.matmul(sps, ones_col, sq,
                                     start=(h == 0), stop=(h == HPC - 1))
                sq2 = rows.tile([1, 2, 512], F32, tag="sq2", name="sq2", bufs=2)
                nc.vector.tensor_copy(sq2[:, 0, :], sps)
                sps_k = pps.tile([1, 512], F32, name="sps_k", tag="ssqp")
                sqk = p1t.tile([128, 512], F32, tag="sq")
                nc.gpsimd.tensor_mul(sqk, kt_n[:, cols], kt_n[:, cols])
                nc.tensor.matmul(sps_k, ones_col, sqk, start=True, stop=True)
                nc.vector.tensor_copy(sq2[:, 1, :], sps_k)
                nc.sync.dma_start(out=ssq_in[t4][:, :, :], in_=sq2)
                nc.gpsimd.collective_compute(
                    "AllReduce", AluOp.add, replica_groups=groups4,
                    ins=[ssq_in[t4].opt()], outs=[ssq_out[t4].opt()])

            def chunk_norm(t4):
                # rstd + broadcast + normalize for chunk t4 (after its AR)
                cols = slice(512 * t4, 512 * (t4 + 1))
                rs_sb = rows.tile([1, 2, 512], F32, tag="rs", name="rs", bufs=2)
                nc.sync.dma_start(out=rs_sb, in_=ssq_out[t4][:, :, :])
                if debug:
                    nc.sync.dma_start(out=dbg["ssq"][:, cols],
                                      in_=rs_sb.rearrange("p r s -> (p r) s"))
                tmp2 = rows.tile([1, 2, 512], F32, tag="sq2", name="tmp", bufs=2)
                rr2 = rows.tile([1, 2, 512], F32, tag="rr", name="rr", bufs=2)
                nc.scalar.activation(out=tmp2[:, 0, :], in_=rs_sb[:, 0, :],
                                     func=AFT.Sqrt, scale=1.0 / (NH * HD),
                                     bias=eps_sb)
                nc.scalar.activation(out=tmp2[:, 1, :], in_=rs_sb[:, 1, :],
                                     func=AFT.Sqrt, scale=1.0 / (NKV * HD),
                                     bias=eps_sb)
                nc.vector.reciprocal(rr2[:, 0, :], tmp2[:, 0, :])
                nc.vector.reciprocal(rr2[:, 1, :], tmp2[:, 1, :])
                bq = pps.tile([128, 512], F32, name="bq", tag="bcq")
                nc.tensor.matmul(bq, ones_row, rr2[:, 0, :],
                                 start=True, stop=True)
                nc.vector.tensor_copy(rq_bc[:, cols], bq)
                bk = pps.tile([128, 512], F32, name="bk", tag="bcq")
                nc.tensor.matmul(bk, ones_row, rr2[:, 1, :],
                                 start=True, stop=True)
                nc.vector.tensor_copy(rk_bc[:, cols], bk)
                for h in range(HPC):
                    nc.vector.scalar_tensor_tensor(
                        out=qt_f[:, h, cols], in0=qt_n[:, h, cols],
                        scalar=wcol_sb[:, h:h + 1], in1=rq_bc[:, cols],
                        op0=AluOp.mult, op1=AluOp.mult)
                nc.vector.tensor_tensor(out=kt_f[:, cols], in0=kt_n[:, cols],
                                        in1=rk_bc[:, cols], op=AluOp.mult)

            prev_xtt = None
            for t4 in range(4):
                cols = slice(512 * t4, 512 * (t4 + 1))
                xtt = load_xtt(cols)
                for h in range(HPC):
                    ps = pp.tile([128, 512], F32)
                    for dt in range(ND):
                        nc.tensor.matmul(
                            ps, wq_sb[:, dt, HD * h:HD * (h + 1)], xtt[dt],
                            start=(dt == 0), stop=(dt == ND - 1))
                    rope_emit(ps, qt_n[:, h], cols)
                ps = pp.tile([128, 512], F32)
                for dt in range(ND):
                    nc.tensor.matmul(ps, wk_sb[:, dt, :], xtt[dt],
                                     start=(dt == 0), stop=(dt == ND - 1))
                rope_emit(ps, kt_n, cols)
                if prev_xtt is not None:
                    v_proj(t4 - 1, prev_xtt)
                ssq_ar(t4)
                if t4 > 0:
                    chunk_norm(t4 - 1)
                prev_xtt = xtt
            v_proj(3, prev_xtt)
            chunk_norm(3)

        if debug:
            nc.sync.dma_start(out=dbg["qt"][:, :, :], in_=qt_f)
            nc.sync.dma_start(out=dbg["kt"][:, :], in_=kt_f)
            nc.sync.dma_start(out=dbg["v"][:, :, :], in_=v_sb)

        # wo prefetch pool: opened after P1 pools close, survives into P5
        wop = outer.enter_context(tc.tile_pool(name="wop", bufs=1))
        wo_sb = wop.tile([128, ND, DIM], BF16)
        nc.sync.dma_start(out=wo_sb,
                          in_=wot_d.rearrange("(n p) e -> p n e", p=128))

        # ---------------- P3: attention (h outer; per-head A2A) ------------
        with ExitStack() as p3s:
            epool = p3s.enter_context(tc.tile_pool(name="epool", bufs=2))
            eraw_p = p3s.enter_context(tc.tile_pool(name="eraw", bufs=3))
            smal = p3s.enter_context(tc.tile_pool(name="smal", bufs=3))
            atp = p3s.enter_context(tc.tile_pool(name="atp", bufs=3))
            stps = p3s.enter_context(
                tc.tile_pool(name="stps", bufs=3, space="PSUM"))
            bcp3 = p3s.enter_context(
                tc.tile_pool(name="bcp3", bufs=1, space="PSUM"))
            pvps = p3s.enter_context(
                tc.tile_pool(name="pvps", bufs=2, space="PSUM"))
            dnps = p3s.enter_context(
                tc.tile_pool(name="dnps", bufs=2, space="PSUM"))
            def attn_epilogue(state):
                # one iteration behind: PE bcast no longer stalls on the
                # reciprocal; next iteration's score matmuls already queued
                h, m, dn_t, at_t, rd_t = state
                bc = bcp3.tile([128, 512], F32, name="bc", tag="bc")
                nc.tensor.matmul(bc, ones_row, rd_t, start=True, stop=True)
                rdb = smal.tile([128, 512], F32, tag="rdb")
                nc.vector.tensor_copy(rdb, bc)
                at0 = atp.tile([128, 512], BF16, tag="at0")
                at1 = atp.tile([128, 512], BF16, tag="at1")
                nc.vector.scalar_tensor_tensor(
                    out=at0, in0=at_t, scalar=bsel_sb[:, 0:1], in1=rdb,
                    op0=AluOp.mult, op1=AluOp.mult)
                nc.vector.scalar_tensor_tensor(
                    out=at1, in0=at_t, scalar=bsel_sb[:, 1:2], in1=rdb,
                    op0=AluOp.mult, op1=AluOp.mult)
                nc.sync.dma_start(out=a2a_in[h][m, :, :], in_=at0)
                nc.sync.dma_start(out=a2a_in[h][TP + m, :, :], in_=at1)

            def attn_epilogue_flush(state):
                attn_epilogue(state)
                if state[1] == 3:
                    hh = state[0]
                    nc.gpsimd.collective_compute(
                        "AllToAll", AluOp.bypass, replica_groups=groups8,
                        ins=[a2a_in[hh].opt()], outs=[a2a_out[hh].opt()])

            pend = None
            for h in range(HPC):
                for m in range(4):
                    qcols = slice(512 * m, 512 * (m + 1))
                    nkb = 4 * m + 4
                    et = epool.tile([128, NT, 512], BF16, tag="e")
                    dn = dnps.tile([1, 512], F32)
                    for kb in range(nkb):
                        st = stps.tile([128, 512], F32, name="st", tag="st")
                        nc.tensor.matmul(
                            st, kt_f[:, 128 * kb:128 * (kb + 1)],
                            qt_f[:, h, qcols], start=True, stop=True)
                        if kb >= 4 * m:
                            o = kb - 4 * m
                            w = 512 - 128 * o
                            er = eraw_p.tile([128, 512], BF16, tag="eraw")
                            nc.scalar.activation(out=er[:, 0:w],
                                                 in_=st[:, 512 - w:512],
                                                 func=AFT.Exp, scale=SCALE)
                            nc.vector.tensor_mul(
                                et[:, kb, 512 - w:512], er[:, 0:w],
                                masks_sb[:, o, 512 - w:512])
                            if o > 0:
                                nc.gpsimd.memset(et[:, kb, 0:512 - w], 0.0)
                        else:
                            nc.scalar.activation(out=et[:, kb, :], in_=st,
                                                 func=AFT.Exp, scale=SCALE)
                    for kb in range(nkb):
                        nc.tensor.matmul(dn, ones_colb, et[:, kb, :],
                                         start=(kb == 0), stop=(kb == nkb - 1))
                    at_ps = pvps.tile([128, 512], F32)
                    for kb in range(nkb):
                        nc.tensor.matmul(at_ps, v_sb[:, kb, :], et[:, kb, :],
                                         start=(kb == 0), stop=(kb == nkb - 1))
                    rd = smal.tile([1, 512], F32, tag="rd")
                    nc.vector.reciprocal(rd, dn)
                    if pend is not None:
                        attn_epilogue_flush(pend)
                    pend = (h, m, dn, at_ps, rd)
            attn_epilogue_flush(pend)

        # ---------------- P5: output projection ----------------
        with ExitStack() as p5s:
            p5 = p5s.enter_context(tc.tile_pool(name="p5", bufs=1))
            p5t = p5s.enter_context(tc.tile_pool(name="p5t", bufs=3))
            pop = p5s.enter_context(
                tc.tile_pool(name="pop", bufs=2, space="PSUM"))
            gt_sb = p5.tile([128, NH, 512], BF16)
            for h in range(HPC):
                for i in range(TP):
                    sA = p5t.tile([128, 512], BF16, tag="sA")
                    sB = p5t.tile([128, 512], BF16, tag="sB")
                    nc.sync.dma_start(out=sA, in_=a2a_out[h][i, :, :])
                    nc.sync.dma_start(out=sB, in_=a2a_out[h][TP + i, :, :])
                    nc.vector.tensor_tensor(out=gt_sb[:, 4 * i + h, :],
                                            in0=sA, in1=sB, op=AluOp.add)
            if debug:
                nc.sync.dma_start(out=dbg["at"][:, :, :], in_=gt_sb)
            e16_order = [4 * i + h for h in range(HPC) for i in range(TP)]
            for tt in range(4):
                opsd = [pop.tile([128, 512], F32, name=f"ops{d}", tag=f"dc{d}")
                        for d in range(4)]
                for idx, e16 in enumerate(e16_order):
                    for dc in range(4):
                        nc.tensor.matmul(
                            opsd[dc], gt_sb[:, e16, 128 * tt:128 * (tt + 1)],
                            wo_sb[:, e16, 512 * dc:512 * (dc + 1)],
                            start=(idx == 0), stop=(idx == NH - 1))
                for dc in range(4):
                    osb = p5t.tile([128, 512], F32, tag="osb")
                    nc.vector.tensor_copy(osb, opsd[dc])
                    nc.sync.dma_start(
                        out=out_d[128 * tt:128 * (tt + 1),
                                  512 * dc:512 * (dc + 1)],
                        in_=osb)

    _split_sync_waits(nc)
    return nc


# ------------------------------------------------------------- host side --
_PERM = np.concatenate([np.arange(0, HD, 2), np.arange(1, HD, 2)])


def _prep(inputs):
    x = np.asarray(inputs["x"], np.float32)
    fc = np.asarray(inputs["freq_cis"], np.float32)
    wq = np.asarray(inputs["wq"], np.float32)
    wk = np.asarray(inputs["wk"], np.float32)
    wv = np.asarray(inputs["wv"], np.float32)
    wo = np.asarray(inputs["wo"], np.float32)
    qnw = np.asarray(inputs["q_norm_w"], np.float32)
    knw = np.asarray(inputs["k_norm_w"], np.float32)

    wq_p = wq.reshape(NH, HD, DIM)[:, _PERM, :]
    wk_p = wk.reshape(NKV, HD, DIM)[:, _PERM, :]
    qnw_p = qnw.reshape(NH, HD)[:, _PERM]
    knw_p = knw.reshape(NKV, HD)[:, _PERM]

    xt = [np.ascontiguousarray(x[b].T).astype(BF) for b in range(B)]
    wqt = [np.ascontiguousarray(
        wq_p[4 * g:4 * (g + 1)].reshape(4 * HD, DIM).T).astype(BF)
        for g in range(TP)]
    wkt = [np.ascontiguousarray(wk_p[g].T).astype(BF) for g in range(TP)]
    wvt = [np.ascontiguousarray(wv[g * HD:(g + 1) * HD].T).astype(BF)
           for g in range(TP)]
    wot = np.ascontiguousarray(wo.T).astype(BF)

    fq = np.stack([fc[:, :, 0, 0].T, fc[:, :, 0, 1].T,
                   fc[:, :, 1, 0].T, fc[:, :, 1, 1].T], axis=1)
    fq = np.concatenate([fq, fq], axis=0)                 # both partition halves
    fq = np.ascontiguousarray(fq).astype(BF)              # [128, 4, S]

    wcol = []
    for g in range(TP):
        cols = np.empty((HD, HPC), np.float32)
        for hl in range(HPC):
            cols[:, hl] = qnw_p[4 * g + hl] * knw_p[g]
        wcol.append(np.ascontiguousarray(cols))

    k_idx = np.arange(128)[:, None]
    q_idx = np.arange(512)[None, :]
    masks = np.stack([(o * 128 + k_idx <= q_idx) for o in range(4)],
                     axis=1).astype(BF)                   # [128, 4, 512]
    masks = np.ascontiguousarray(masks)

    in_maps = []
    for c in range(8):
        b, g = divmod(c, TP)
        bsel = np.zeros((128, 2), np.float32)
        bsel[:, b] = 1.0
        in_maps.append({
            "xt": xt[b], "wqt": wqt[g], "wkt": wkt[g], "wvt": wvt[g],
            "wot": wot, "fq": fq, "wcol": wcol[g], "masks": masks,
            "bsel": bsel,
        })
    return in_maps


_GRAPH_CACHE = {}


def _get_graph(debug=False):
    key = bool(debug)
    if key not in _GRAPH_CACHE:
        _GRAPH_CACHE[key] = build_graph(debug=key)
    return _GRAPH_CACHE[key]


LAST_RESULT = None


def kernel(debug=False, _run_kwargs=None, **inputs):
    global LAST_RESULT
    from concourse.bass_utils import run_bass_kernel_spmd

    nc = _get_graph(debug=debug)
    in_maps = _prep(inputs)
    res = run_bass_kernel_spmd(nc, in_maps, core_ids=list(range(8)),
                               **(_run_kwargs or {}))
    LAST_RESULT = res
    out = np.empty((B, S, DIM), np.float32)
    for c in range(8):
        b, g = divmod(c, TP)
        out[b, TOK * g:TOK * (g + 1), :] = res.results[c]["out"]
    if debug:
        return out, res
    return out
